# revision 2
# baseline (speedup 1.0000x reference)
"""MoE (2-expert SwiGLU) Trainium2 kernel, 8-core SPMD.

Strategy: since the MLPs have no biases and silu(0) = 0, MLP(0) = 0, so each
token only needs the expert it is routed to.  The host gathers tokens by
expert (MoE dispatch), cores 0-3 process expert-0 tokens and cores 4-7
expert-1 tokens (~1/8 of total tokens per core), each core running a dense
SwiGLU MLP with its expert's weights.  The host scatters per-core outputs
back into the full [B, S, D] output.  This halves FLOPs vs. the reference's
dense-masked formulation and needs no collectives.

Device dataflow (per core, transposed so no on-chip transposes are needed):
  yT = Wd^T @ (silu(Wg^T @ xT) * (Wu^T @ xT))
Weights are the stationary matmul operand, token-columns the moving operand.
All matmuls are bf16 with fp32 PSUM accumulation.  The FF intermediate `h`
for all of a core's tokens stays resident in SBUF, so each weight byte is
DMA'd exactly once per core.

Schedule notes (from trace analysis):
 - The steady-state matmul cadence is at the HW limit (N/2.4GHz + 2.5ns),
   so the remaining wins are at the edges: kernel startup, the stage-1 ->
   stage-2 transition, and the epilogue.
 - Startup: the first matmul only needs Wg/Wu k-slices kd=0..1 and the x
   k-slice kd=0, so those are DMA'd as small leading chunks.  x + the first
   weight chunk go on the Sync HWDGE ring; the bulk weight stream goes on
   the Activation HWDGE ring so the two don't serialize behind each other.
 - Both stages share one PSUM tile pool; stage 2 allocates its accumulators
   from the same tags stage 1 used, so buffer-slot reuse is tracked
   per-slot (no pool-close barrier between the stages).
"""

import sys

for _p in ("/opt/trn_rl_repo", "/root/.axon_site/_ro/trn_rl_repo"):
    if _p not in sys.path:
        sys.path.append(_p)

import numpy as np
import ml_dtypes

BF16 = ml_dtypes.bfloat16

D_MODEL = 1024
D_FF = 4096
P = 128
KD = D_MODEL // P  # 8   k-tiles over d_model
MF = D_FF // P     # 32  tiles over d_ff
N_CORES = 8
CPE = 4            # cores per expert
NT = 3             # token tiles per core

_program_cache: dict[tuple, object] = {}


def _token_tiles(maxpc: int) -> tuple:
    """Split the per-core token count into NT near-equal tiles.  Tile
    offsets stay multiples of 8 (16B-aligned bf16 slices); the last tile
    absorbs the remainder.  Each tile <= 512 (PSUM bank limit)."""
    C = max(maxpc, 24)
    t = 8 * ((C + NT * 8 - 1) // (NT * 8))    # per-tile, rounded up to 8
    tiles = [t] * (NT - 1) + [C - t * (NT - 1)]
    assert all(0 < s <= 512 for s in tiles), (maxpc, tiles)
    return tuple(tiles)


def _build_program(tiles: tuple):
    """Bass program for one core: x [D,C] -> y [D,C], C = sum(tiles) tokens."""
    import concourse.tile as tile
    from concourse import mybir, bacc

    C = sum(tiles)
    offs = [sum(tiles[:i]) for i in range(len(tiles))]
    TSMAX = max(tiles)
    f32 = mybir.dt.float32
    b16 = mybir.dt.bfloat16

    nc = bacc.Bacc()
    xT = nc.declare_dram_parameter("xT", [P, KD, C], b16, isOutput=False)
    # w1[mf, p, kd, gu, c] = (wg if gu==0 else wu)[kd*128 + p, mf*128 + c]
    # kd-major so the startup chunk w1[0][:, 0:2] is contiguous.
    w1 = nc.declare_dram_parameter("w1", [MF, P, KD, 2, P], b16, isOutput=False)
    # wdp[md, p, kf, c] = wd[kf*128 + p, md*128 + c]
    wdp = nc.declare_dram_parameter("wd", [KD, P, MF, P], b16, isOutput=False)
    yT = nc.declare_dram_parameter("yT", [KD, P, C], b16, isOutput=True)

    with tile.TileContext(nc) as tc:
        with (
            tc.tile_pool(name="iop", bufs=1) as iop,
            tc.tile_pool(name="wp", bufs=3) as wp,
            tc.tile_pool(name="workp", bufs=2) as workp,
            tc.tile_pool(name="ps", bufs=NT, space="PSUM") as ps,
        ):
            x_sb = iop.tile([P, KD, C], b16, tag="x")
            h_sb = iop.tile([P, MF, C], b16, tag="h")
            wt0 = wp.tile([P, KD, 2, P], b16, tag="wt", name="wt_0")
            # Startup: the first matmuls need wt0 kd=0..1 and x kd=0, so
            # those lead.  Sync ring: x + the small first weight chunk.
            # Act ring: the bulk weight stream (wt0 tail + wt_1..wt_31),
            # so it never queues behind the 2 MB x transfer.
            nc.sync.dma_start(wt0[:, 0:2], w1[0, :, 0:2])
            nc.sync.dma_start(x_sb[:, 0], xT[:, 0])
            nc.sync.dma_start(x_sb[:, 1], xT[:, 1])
            nc.sync.dma_start(x_sb[:, 2], xT[:, 2])
            nc.sync.dma_start(x_sb[:, 3], xT[:, 3])
            nc.sync.dma_start(x_sb[:, 4:6], xT[:, 4:6])
            nc.sync.dma_start(x_sb[:, 6:], xT[:, 6:])
            nc.scalar.dma_start(wt0[:, 2:], w1[0, :, 2:])

            # Stage 1: h = silu(Wg^T x) * (Wu^T x), laid out [ff-part, C]
            for mf in range(MF):
                if mf == 0:
                    wt = wt0
                else:
                    wt = wp.tile([P, KD, 2, P], b16, tag="wt",
                                 name=f"wt_{mf}")
                    nc.scalar.dma_start(wt[:], w1[mf])
                psg = [ps.tile([P, 512], f32, tag="psg", name=f"psg_{mf}_{t}")
                       for t in range(NT)]
                psu = [ps.tile([P, 512], f32, tag="psu", name=f"psu_{mf}_{t}")
                       for t in range(NT)]
                for kd in range(KD):
                    for gu in range(2):
                        psx = psg if gu == 0 else psu
                        for t in range(NT):
                            nc.tensor.matmul(
                                psx[t][:, :tiles[t]],
                                wt[:, kd, gu],
                                x_sb[:, kd, offs[t]:offs[t] + tiles[t]],
                                start=(kd == 0),
                                stop=(kd == KD - 1),
                            )
                for t in range(NT):
                    sil = workp.tile([P, TSMAX], f32, tag="sil", bufs=4,
                                     name=f"sil_{mf}_{t}")
                    nc.scalar.activation(
                        sil[:, :tiles[t]], psg[t][:, :tiles[t]],
                        mybir.ActivationFunctionType.Silu,
                    )
                    nc.vector.tensor_mul(
                        h_sb[:, mf, offs[t]:offs[t] + tiles[t]],
                        sil[:, :tiles[t]], psu[t][:, :tiles[t]],
                    )

            # Stage 2: y = Wd^T h, laid out [d-part, C].  PSUM accumulators
            # come from the same pool/tags as stage 1, so each one only
            # waits for its own slot's last reader (no stage barrier).
            for md in range(KD):
                wdt = wp.tile([P, MF, P], b16, tag="wd", bufs=2,
                              name=f"wd_{md}")
                nc.sync.dma_start(wdt[:], wdp[md])
                y_sb = workp.tile([P, C], b16, tag="y", name=f"y_{md}")
                ptag = "psg" if md % 2 == 0 else "psu"
                psy = [ps.tile([P, 512], f32, tag=ptag, name=f"psy_{md}_{t}")
                       for t in range(NT)]
                for kf in range(MF):
                    for t in range(NT):
                        nc.tensor.matmul(
                            psy[t][:, :tiles[t]],
                            wdt[:, kf],
                            h_sb[:, kf, offs[t]:offs[t] + tiles[t]],
                            start=(kf == 0),
                            stop=(kf == MF - 1),
                        )
                for t in range(NT):
                    nc.vector.tensor_copy(
                        y_sb[:, offs[t]:offs[t] + tiles[t]],
                        psy[t][:, :tiles[t]],
                    )
                    # per-tile store so the final DMA after the last
                    # matmul is small
                    nc.sync.dma_start(
                        yT[md, :, offs[t]:offs[t] + tiles[t]],
                        y_sb[:, offs[t]:offs[t] + tiles[t]],
                    )

    nc.compile()
    return nc


def _pack_w1(wg: np.ndarray, wu: np.ndarray) -> np.ndarray:
    """[D, F] x2 -> [MF, P, KD, 2, P] bf16, matching the kernel's layout."""
    # w1[mf, p, kd, gu, c] = w_gu[kd*128 + p, mf*128 + c]
    stack = np.stack([wg, wu], axis=0)            # [2, D, F]
    r = stack.reshape(2, KD, P, MF, P)            # [gu, kd, p, mf, c]
    return np.ascontiguousarray(r.transpose(3, 2, 1, 0, 4)).astype(BF16)


def _pack_wd(wd: np.ndarray) -> np.ndarray:
    """[F, D] -> [KD, P, MF, P] bf16. wdp[md, p, kf, c] = wd[kf*128+p, md*128+c]"""
    r = wd.reshape(MF, P, KD, P)                  # [kf, p, md, c]
    return np.ascontiguousarray(r.transpose(2, 1, 0, 3)).astype(BF16)


def _run_device(in_maps, tiles):
    from concourse.bass_utils import run_bass_kernel_spmd

    if tiles not in _program_cache:
        _program_cache[tiles] = _build_program(tiles)
    nc = _program_cache[tiles]
    res = run_bass_kernel_spmd(nc, in_maps, core_ids=list(range(N_CORES)))
    return [r["yT"] for r in res.results]


def kernel(hidden_states, routing_mask, wg0, wu0, wd0, wg1, wu1, wd1,
           _run=None):
    hidden_states = np.asarray(hidden_states, dtype=np.float32)
    routing_mask = np.asarray(routing_mask)
    B, S, D = hidden_states.shape
    NTOK = B * S
    x = hidden_states.reshape(NTOK, D)
    mask = routing_mask.reshape(NTOK)

    idx = [np.nonzero(mask == e)[0] for e in (0, 1)]
    maxpc = max(
        (len(idx[0]) + CPE - 1) // CPE,
        (len(idx[1]) + CPE - 1) // CPE,
        1,
    )
    tiles = _token_tiles(maxpc)
    C = sum(tiles)

    w1_packed = [_pack_w1(np.asarray(wg0), np.asarray(wu0)),
                 _pack_w1(np.asarray(wg1), np.asarray(wu1))]
    wd_packed = [_pack_wd(np.asarray(wd0)), _pack_wd(np.asarray(wd1))]

    in_maps = []
    chunks = []  # (expert, token_indices) per core
    for core in range(N_CORES):
        e = core // CPE
        slot = core % CPE
        ids = idx[e]
        # split ids into CPE nearly-equal chunks
        bounds = [(len(ids) * i) // CPE for i in range(CPE + 1)]
        ids_c = ids[bounds[slot]:bounds[slot + 1]]
        chunks.append((e, ids_c))

        xc = np.zeros((C, D), dtype=np.float32)
        xc[: len(ids_c)] = x[ids_c]
        # xT[p, kd, c] = xc[c, kd*128 + p]
        xT = np.ascontiguousarray(
            xc.reshape(C, KD, P).transpose(2, 1, 0)
        ).astype(BF16)
        in_maps.append({
            "xT": xT,
            "w1": w1_packed[e],
            "wd": wd_packed[e],
        })

    run = _run if _run is not None else _run_device
    outs = run(in_maps, tiles)

    y_full = np.zeros((NTOK, D), dtype=np.float32)
    for core in range(N_CORES):
        _, ids_c = chunks[core]
        if len(ids_c) == 0:
            continue
        yT = np.asarray(outs[core]).astype(np.float32).reshape(D, C)
        y_full[ids_c] = yT[:, : len(ids_c)].T
    return y_full.reshape(B, S, D)


# revision 3
# speedup vs baseline: 1.0005x; 1.0005x over previous
"""MoE (2-expert SwiGLU) Trainium2 kernel, 8-core SPMD.

Strategy: since the MLPs have no biases and silu(0) = 0, MLP(0) = 0, so each
token only needs the expert it is routed to.  The host gathers tokens by
expert (MoE dispatch), cores 0-3 process expert-0 tokens and cores 4-7
expert-1 tokens (~1/8 of total tokens per core), each core running a dense
SwiGLU MLP with its expert's weights.  The host scatters per-core outputs
back into the full [B, S, D] output.  This halves FLOPs vs. the reference's
dense-masked formulation and needs no collectives.

Device dataflow (per core, transposed so no on-chip transposes are needed):
  yT = Wd^T @ (silu(Wg^T @ xT) * (Wu^T @ xT))
Weights are the stationary matmul operand, token-columns the moving operand.
All matmuls are bf16 with fp32 PSUM accumulation.  The FF intermediate `h`
for all of a core's tokens stays resident in SBUF, so each weight byte is
DMA'd exactly once per core.

Schedule notes (from trace analysis; steady-state matmul cadence is already
at the HW limit of N/2.4GHz + ~2.5ns, so the wins are at the edges):
 - Warm-up: ~3us of tiny matmuls on scratch data run while the startup DMAs
   are in flight, so the PE_HAM clock gate un-throttles (1.2 -> 2.4 GHz)
   before the real matmuls start.
 - Startup: the first matmuls need only x kd=0 and Wg/Wu kd=0..1, so those
   are DMA'd as small leading chunks, split across both HWDGE rings (Sync +
   Activation) because DMA completion has a ~2.4us receipt latency and one
   ring alone can't feed the first mf pass.
 - The first LDWEIGHTS of each freshly-DMA'd weight tile stalls the PE
   ~80ns (its DMA-semaphore wait defeats the load-ahead), so stage-1
   weights are fetched in 2-mf pairs to halve the number of boundaries.
 - Both stages share one PSUM tile pool; stage 2 allocates its accumulators
   from the same tags stage 1 used, so buffer-slot reuse is tracked
   per-slot (no pool-close barrier).  wd tiles share the "wt" tag, which
   both reuses its SBUF slots and naturally times the wd prefetch.
"""

import sys

for _p in ("/opt/trn_rl_repo", "/root/.axon_site/_ro/trn_rl_repo"):
    if _p not in sys.path:
        sys.path.append(_p)

import numpy as np
import ml_dtypes

BF16 = ml_dtypes.bfloat16

D_MODEL = 1024
D_FF = 4096
P = 128
KD = D_MODEL // P  # 8   k-tiles over d_model
MF = D_FF // P     # 32  tiles over d_ff
N_CORES = 8
CPE = 4            # cores per expert
NT = 3             # token tiles per core
N_WARM = 60        # HAM warm-up matmuls

_program_cache: dict[tuple, object] = {}


def _token_tiles(maxpc: int) -> tuple:
    """Split the per-core token count into NT near-equal tiles.  Tile
    offsets stay multiples of 8 (16B-aligned bf16 slices); the last tile
    absorbs the remainder.  Each tile <= 512 (PSUM bank limit)."""
    C = max(maxpc, 24)
    t = 8 * ((C + NT * 8 - 1) // (NT * 8))    # per-tile, rounded up to 8
    tiles = [t] * (NT - 1) + [C - t * (NT - 1)]
    assert all(0 < s <= 512 for s in tiles), (maxpc, tiles)
    return tuple(tiles)


def _build_program(tiles: tuple):
    """Bass program for one core: x [D,C] -> y [D,C], C = sum(tiles) tokens."""
    import concourse.tile as tile
    from concourse import mybir, bacc

    C = sum(tiles)
    offs = [sum(tiles[:i]) for i in range(len(tiles))]
    TSMAX = max(tiles)
    f32 = mybir.dt.float32
    b16 = mybir.dt.bfloat16

    nc = bacc.Bacc()
    xT = nc.declare_dram_parameter("xT", [P, KD, C], b16, isOutput=False)
    # w1[p, mf, kd, gu, c] = (wg if gu==0 else wu)[kd*128 + p, mf*128 + c]
    # partition-major so any mf-group slice is one contiguous-per-partition
    # DMA (one DMA per weight tile keeps the boundary LDWEIGHTS waits rare).
    w1 = nc.declare_dram_parameter("w1", [P, MF, KD, 2, P], b16, isOutput=False)
    # wdp[p, md, kf, c] = wd[kf*128 + p, md*128 + c]
    wdp = nc.declare_dram_parameter("wd", [P, KD, MF, P], b16, isOutput=False)
    yT = nc.declare_dram_parameter("yT", [KD, P, C], b16, isOutput=True)

    # stage-1 weight fetch groups: mf 0 and 1 alone (startup-critical),
    # then pairs
    groups = [(0,), (1,)] + [(2 + 2 * i, 3 + 2 * i) for i in range(15)]

    with tile.TileContext(nc) as tc:
        with (
            tc.tile_pool(name="iop", bufs=1) as iop,
            tc.tile_pool(name="wp", bufs=3) as wp,
            tc.tile_pool(name="workp", bufs=2) as workp,
            tc.tile_pool(name="ps", bufs=NT, space="PSUM") as ps,
        ):
            x_sb = iop.tile([P, KD, C], b16, tag="x")
            h_sb = iop.tile([P, MF, C], b16, tag="h")

            # HAM warm-up: keep the PE busy on scratch data while the
            # startup DMAs fly, so the clock gate opens before real work.
            warm_x = workp.tile([P, 16], b16, tag="warm", bufs=1)
            nc.gpsimd.memset(warm_x[:], 0.0)
            pw = ps.tile([P, 16], f32, tag="warm", bufs=1, name="pw")
            for i in range(N_WARM):
                nc.tensor.matmul(pw[:16, :16], warm_x[:, :16], warm_x[:, :16],
                                 start=True, stop=True)

            wt0 = wp.tile([P, KD, 2, P], b16, tag="wt", name="wt_0")
            # Startup chunks.  Sync ring: x kd 0-3 and 7 plus Wg/Wu kd 0-1;
            # Act ring: Wg/Wu kd 2-7, x kd 4-6, then the weight stream.
            nc.sync.dma_start(x_sb[:, 0, :offs[1]], xT[:, 0, :offs[1]])
            nc.sync.dma_start(wt0[:, 0:1], w1[:, 0, 0:1])
            nc.sync.dma_start(x_sb[:, 0, offs[1]:], xT[:, 0, offs[1]:])
            nc.sync.dma_start(wt0[:, 1:2], w1[:, 0, 1:2])
            nc.sync.dma_start(x_sb[:, 1], xT[:, 1])
            nc.sync.dma_start(x_sb[:, 2], xT[:, 2])
            nc.sync.dma_start(x_sb[:, 3], xT[:, 3])
            nc.sync.dma_start(x_sb[:, 7], xT[:, 7])
            nc.scalar.dma_start(wt0[:, 2:], w1[:, 0, 2:])
            nc.scalar.dma_start(x_sb[:, 4], xT[:, 4])
            nc.scalar.dma_start(x_sb[:, 5], xT[:, 5])
            nc.scalar.dma_start(x_sb[:, 6], xT[:, 6])

            # Stage 1: h = silu(Wg^T x) * (Wu^T x), laid out [ff-part, C]
            for gi, g in enumerate(groups):
                if gi == 0:
                    wt = wt0
                else:
                    wt = wp.tile([P, len(g), KD, 2, P], b16, tag="wt",
                                 name=f"wt_{g[0]}")
                    nc.scalar.dma_start(wt[:], w1[:, g[0]:g[-1] + 1])
                for j, mf in enumerate(g):
                    wtj = wt if gi == 0 else wt[:, j]
                    psg = [ps.tile([P, 512], f32, tag="psg",
                                   name=f"psg_{mf}_{t}") for t in range(NT)]
                    psu = [ps.tile([P, 512], f32, tag="psu",
                                   name=f"psu_{mf}_{t}") for t in range(NT)]
                    for kd in range(KD):
                        for gu in range(2):
                            psx = psg if gu == 0 else psu
                            for t in range(NT):
                                nc.tensor.matmul(
                                    psx[t][:, :tiles[t]],
                                    wtj[:, kd, gu],
                                    x_sb[:, kd, offs[t]:offs[t] + tiles[t]],
                                    start=(kd == 0),
                                    stop=(kd == KD - 1),
                                )
                    for t in range(NT):
                        sil = workp.tile([P, TSMAX], f32, tag="sil", bufs=4,
                                         name=f"sil_{mf}_{t}")
                        nc.scalar.activation(
                            sil[:, :tiles[t]], psg[t][:, :tiles[t]],
                            mybir.ActivationFunctionType.Silu,
                        )
                        nc.vector.tensor_mul(
                            h_sb[:, mf, offs[t]:offs[t] + tiles[t]],
                            sil[:, :tiles[t]], psu[t][:, :tiles[t]],
                        )

            # Stage 2: y = Wd^T h, laid out [d-part, C].  PSUM accumulators
            # come from the same pool/tags as stage 1, so each waits only
            # for its own slot's last reader (no stage barrier).  wd tiles
            # share the "wt" tag: the slot dependency releases each wd DMA
            # a few mf-iterations before stage 2 needs it.
            for md in range(KD):
                wdt = wp.tile([P, MF, P], b16, tag="wt", name=f"wd_{md}")
                nc.sync.dma_start(wdt[:], wdp[:, md])
                y_sb = workp.tile([P, C], b16, tag="y", name=f"y_{md}")
                ptag = "psg" if md % 2 == 0 else "psu"
                psy = [ps.tile([P, 512], f32, tag=ptag, name=f"psy_{md}_{t}")
                       for t in range(NT)]
                for kf in range(MF):
                    for t in range(NT):
                        nc.tensor.matmul(
                            psy[t][:, :tiles[t]],
                            wdt[:, kf],
                            h_sb[:, kf, offs[t]:offs[t] + tiles[t]],
                            start=(kf == 0),
                            stop=(kf == MF - 1),
                        )
                for t in range(NT):
                    nc.vector.tensor_copy(
                        y_sb[:, offs[t]:offs[t] + tiles[t]],
                        psy[t][:, :tiles[t]],
                    )
                    # per-tile store (act ring: free in stage 2, and the
                    # final DMA after the last matmul stays small)
                    nc.scalar.dma_start(
                        yT[md, :, offs[t]:offs[t] + tiles[t]],
                        y_sb[:, offs[t]:offs[t] + tiles[t]],
                    )

    nc.compile()
    return nc


def _pack_w1(wg: np.ndarray, wu: np.ndarray) -> np.ndarray:
    """[D, F] x2 -> [P, MF, KD, 2, P] bf16, matching the kernel's layout."""
    # w1[p, mf, kd, gu, c] = w_gu[kd*128 + p, mf*128 + c]
    stack = np.stack([wg, wu], axis=0)            # [2, D, F]
    r = stack.reshape(2, KD, P, MF, P)            # [gu, kd, p, mf, c]
    return np.ascontiguousarray(r.transpose(2, 3, 1, 0, 4)).astype(BF16)


def _pack_wd(wd: np.ndarray) -> np.ndarray:
    """[F, D] -> [P, KD, MF, P] bf16. wdp[p, md, kf, c] = wd[kf*128+p, md*128+c]"""
    r = wd.reshape(MF, P, KD, P)                  # [kf, p, md, c]
    return np.ascontiguousarray(r.transpose(1, 2, 0, 3)).astype(BF16)


def _run_device(in_maps, tiles):
    from concourse.bass_utils import run_bass_kernel_spmd

    if tiles not in _program_cache:
        _program_cache[tiles] = _build_program(tiles)
    nc = _program_cache[tiles]
    res = run_bass_kernel_spmd(nc, in_maps, core_ids=list(range(N_CORES)))
    return [r["yT"] for r in res.results]


def kernel(hidden_states, routing_mask, wg0, wu0, wd0, wg1, wu1, wd1,
           _run=None):
    hidden_states = np.asarray(hidden_states, dtype=np.float32)
    routing_mask = np.asarray(routing_mask)
    B, S, D = hidden_states.shape
    NTOK = B * S
    x = hidden_states.reshape(NTOK, D)
    mask = routing_mask.reshape(NTOK)

    idx = [np.nonzero(mask == e)[0] for e in (0, 1)]
    maxpc = max(
        (len(idx[0]) + CPE - 1) // CPE,
        (len(idx[1]) + CPE - 1) // CPE,
        1,
    )
    tiles = _token_tiles(maxpc)
    C = sum(tiles)

    w1_packed = [_pack_w1(np.asarray(wg0), np.asarray(wu0)),
                 _pack_w1(np.asarray(wg1), np.asarray(wu1))]
    wd_packed = [_pack_wd(np.asarray(wd0)), _pack_wd(np.asarray(wd1))]

    in_maps = []
    chunks = []  # (expert, token_indices) per core
    for core in range(N_CORES):
        e = core // CPE
        slot = core % CPE
        ids = idx[e]
        # split ids into CPE nearly-equal chunks
        bounds = [(len(ids) * i) // CPE for i in range(CPE + 1)]
        ids_c = ids[bounds[slot]:bounds[slot + 1]]
        chunks.append((e, ids_c))

        xc = np.zeros((C, D), dtype=np.float32)
        xc[: len(ids_c)] = x[ids_c]
        # xT[p, kd, c] = xc[c, kd*128 + p]
        xT = np.ascontiguousarray(
            xc.reshape(C, KD, P).transpose(2, 1, 0)
        ).astype(BF16)
        in_maps.append({
            "xT": xT,
            "w1": w1_packed[e],
            "wd": wd_packed[e],
        })

    run = _run if _run is not None else _run_device
    outs = run(in_maps, tiles)

    y_full = np.zeros((NTOK, D), dtype=np.float32)
    for core in range(N_CORES):
        _, ids_c = chunks[core]
        if len(ids_c) == 0:
            continue
        yT = np.asarray(outs[core]).astype(np.float32).reshape(D, C)
        y_full[ids_c] = yT[:, : len(ids_c)].T
    return y_full.reshape(B, S, D)


# revision 6
# speedup vs baseline: 1.0016x; 1.0011x over previous
"""MoE (2-expert SwiGLU) Trainium2 kernel, 8-core SPMD.

Strategy: since the MLPs have no biases and silu(0) = 0, MLP(0) = 0, so each
token only needs the expert it is routed to.  The host gathers tokens by
expert (MoE dispatch), cores 0-3 process expert-0 tokens and cores 4-7
expert-1 tokens (~1/8 of total tokens per core), each core running a dense
SwiGLU MLP with its expert's weights.  The host scatters per-core outputs
back into the full [B, S, D] output.  This halves FLOPs vs. the reference's
dense-masked formulation and needs no collectives.

Device dataflow (per core, transposed so no on-chip transposes are needed):
  yT = Wd^T @ (silu(Wg^T @ xT) * (Wu^T @ xT))
Weights are the stationary matmul operand, token-columns the moving operand.
All matmuls are bf16 with fp32 PSUM accumulation.  The FF intermediate `h`
for all of a core's tokens stays resident in SBUF, so each weight byte is
DMA'd exactly once per core.

Schedule notes (from trace analysis; steady-state matmul cadence is already
at the HW limit of N/2.4GHz + ~2.5ns, so the wins are at the edges):
 - Warm-up: ~3us of tiny matmuls on scratch data run while the startup DMAs
   are in flight, so the PE_HAM clock gate un-throttles (1.2 -> 2.4 GHz)
   before the real matmuls start.
 - Startup: the first matmuls need only x kd=0 and Wg/Wu kd=0..1, so those
   are DMA'd as small leading chunks, split across both HWDGE rings (Sync +
   Activation) because DMA completion has a ~2.4us receipt latency and one
   ring alone can't feed the first mf pass.
 - The first LDWEIGHTS of each freshly-DMA'd weight tile stalls the PE
   ~80ns (its DMA-semaphore wait defeats the load-ahead), so stage-1
   weights are fetched in 2-mf pairs to halve the number of boundaries.
 - Both stages share one PSUM tile pool; stage 2 allocates its accumulators
   from the same tags stage 1 used, so buffer-slot reuse is tracked
   per-slot (no pool-close barrier).  wd tiles share the "wt" tag, which
   both reuses its SBUF slots and naturally times the wd prefetch.
"""

import sys

for _p in ("/opt/trn_rl_repo", "/root/.axon_site/_ro/trn_rl_repo"):
    if _p not in sys.path:
        sys.path.append(_p)

import numpy as np
import ml_dtypes

BF16 = ml_dtypes.bfloat16

D_MODEL = 1024
D_FF = 4096
P = 128
KD = D_MODEL // P  # 8   k-tiles over d_model
MF = D_FF // P     # 32  tiles over d_ff
N_CORES = 8
CPE = 4            # cores per expert
NT = 3             # token tiles per core
N_WARM = 34        # HAM warm-up matmuls (N=128 each, ~3.6us cold)

_program_cache: dict[tuple, object] = {}


def _token_tiles(maxpc: int) -> tuple:
    """Split the per-core token count into NT near-equal tiles.  Tile
    offsets stay multiples of 8 (16B-aligned bf16 slices); the last tile
    absorbs the remainder.  Each tile <= 512 (PSUM bank limit)."""
    C = max(maxpc, 24)
    t = 8 * ((C + NT * 8 - 1) // (NT * 8))    # per-tile, rounded up to 8
    tiles = [t] * (NT - 1) + [C - t * (NT - 1)]
    assert all(0 < s <= 512 for s in tiles), (maxpc, tiles)
    return tuple(tiles)


def _build_program(tiles: tuple):
    """Bass program for one core: x [D,C] -> y [D,C], C = sum(tiles) tokens."""
    import concourse.tile as tile
    from concourse import mybir, bacc

    C = sum(tiles)
    offs = [sum(tiles[:i]) for i in range(len(tiles))]
    TSMAX = max(tiles)
    f32 = mybir.dt.float32
    b16 = mybir.dt.bfloat16

    nc = bacc.Bacc(num_swdge_queues=4)
    xT = nc.declare_dram_parameter("xT", [P, KD, C], b16, isOutput=False)
    # w1[p, mf, kd, gu, c] = (wg if gu==0 else wu)[kd*128 + p, mf*128 + c]
    # partition-major so any mf-group slice is one contiguous-per-partition
    # DMA (one DMA per weight tile keeps the boundary LDWEIGHTS waits rare).
    w1 = nc.declare_dram_parameter("w1", [P, MF, KD, 2, P], b16, isOutput=False)
    # wdp[p, md, kf, c] = wd[kf*128 + p, md*128 + c]
    wdp = nc.declare_dram_parameter("wd", [P, KD, MF, P], b16, isOutput=False)
    yT = nc.declare_dram_parameter("yT", [KD, P, C], b16, isOutput=True)

    # stage-1 weight fetch groups: mf 0 and 1 alone (startup-critical),
    # then pairs
    groups = [(0,), (1,)] + [(2 + 2 * i, 3 + 2 * i) for i in range(15)]

    with tile.TileContext(nc) as tc:
        with (
            tc.tile_pool(name="iop", bufs=1) as iop,
            tc.tile_pool(name="wp", bufs=3) as wp,
            tc.tile_pool(name="workp", bufs=2) as workp,
            tc.tile_pool(name="ps", bufs=NT, space="PSUM") as ps,
        ):
            x_sb = iop.tile([P, KD, C], b16, tag="x")
            h_sb = iop.tile([P, MF, C], b16, tag="h")

            # HAM warm-up: keep the PE busy on scratch data while the
            # startup DMAs fly, so the clock gate opens before real work.
            warm_x = workp.tile([P, P], b16, tag="warm", bufs=1)
            nc.vector.memset(warm_x[:], 0.0)
            pw = ps.tile([P, P], f32, tag="warm", bufs=1, name="pw")
            for i in range(N_WARM):
                nc.tensor.matmul(pw[:], warm_x[:], warm_x[:],
                                 start=True, stop=True)

            wt0 = wp.tile([P, KD, 2, P], b16, tag="wt", name="wt_0")
            # Startup chunks.  The first matmuls need x kd0 + Wg/Wu kd0-1;
            # those ride SWDGE (gpsimd) queues, whose ~2us fixed cost beats
            # the HWDGE rings' ~3us completion pipeline.  The rest of x is
            # split across both HWDGE rings; the bulk weight stream follows
            # on the Act ring.
            nc.gpsimd.dma_start(x_sb[:, 0], xT[:, 0])
            nc.gpsimd.dma_start(wt0[:, 0:2], w1[:, 0, 0:2])
            nc.gpsimd.dma_start(x_sb[:, 1], xT[:, 1])
            nc.sync.dma_start(x_sb[:, 2], xT[:, 2])
            nc.sync.dma_start(x_sb[:, 3], xT[:, 3])
            nc.sync.dma_start(x_sb[:, 7], xT[:, 7])
            nc.scalar.dma_start(wt0[:, 2:], w1[:, 0, 2:])
            nc.scalar.dma_start(x_sb[:, 4], xT[:, 4])
            nc.scalar.dma_start(x_sb[:, 5], xT[:, 5])
            nc.scalar.dma_start(x_sb[:, 6], xT[:, 6])

            # Stage 1: h = silu(Wg^T x) * (Wu^T x), laid out [ff-part, C]
            for gi, g in enumerate(groups):
                if gi == 0:
                    wt = wt0
                else:
                    wt = wp.tile([P, len(g), KD, 2, P], b16, tag="wt",
                                 name=f"wt_{g[0]}")
                    nc.scalar.dma_start(wt[:], w1[:, g[0]:g[-1] + 1])
                for j, mf in enumerate(g):
                    wtj = wt if gi == 0 else wt[:, j]
                    psg = [ps.tile([P, 512], f32, tag="psg",
                                   name=f"psg_{mf}_{t}") for t in range(NT)]
                    psu = [ps.tile([P, 512], f32, tag="psu",
                                   name=f"psu_{mf}_{t}") for t in range(NT)]
                    for kd in range(KD):
                        for gu in range(2):
                            psx = psg if gu == 0 else psu
                            for t in range(NT):
                                nc.tensor.matmul(
                                    psx[t][:, :tiles[t]],
                                    wtj[:, kd, gu],
                                    x_sb[:, kd, offs[t]:offs[t] + tiles[t]],
                                    start=(kd == 0),
                                    stop=(kd == KD - 1),
                                )
                    for t in range(NT):
                        sil = workp.tile([P, TSMAX], f32, tag="sil", bufs=4,
                                         name=f"sil_{mf}_{t}")
                        nc.scalar.activation(
                            sil[:, :tiles[t]], psg[t][:, :tiles[t]],
                            mybir.ActivationFunctionType.Silu,
                        )
                        nc.vector.tensor_mul(
                            h_sb[:, mf, offs[t]:offs[t] + tiles[t]],
                            sil[:, :tiles[t]], psu[t][:, :tiles[t]],
                        )

            # Stage 2: y = Wd^T h, laid out [d-part, C].  PSUM accumulators
            # come from the same pool/tags as stage 1, so each waits only
            # for its own slot's last reader (no stage barrier).  wd tiles
            # share the "wt" tag: the slot dependency releases each wd DMA
            # a few mf-iterations before stage 2 needs it.
            for md in range(KD):
                wdt = wp.tile([P, MF, P], b16, tag="wt", name=f"wd_{md}")
                nc.sync.dma_start(wdt[:], wdp[:, md])
                y_sb = workp.tile([P, C], b16, tag="y", name=f"y_{md}")
                ptag = "psg" if md % 2 == 0 else "psu"
                psy = [ps.tile([P, 512], f32, tag=ptag, name=f"psy_{md}_{t}")
                       for t in range(NT)]
                for kf in range(MF):
                    for t in range(NT):
                        nc.tensor.matmul(
                            psy[t][:, :tiles[t]],
                            wdt[:, kf],
                            h_sb[:, kf, offs[t]:offs[t] + tiles[t]],
                            start=(kf == 0),
                            stop=(kf == MF - 1),
                        )
                for t in range(NT):
                    nc.vector.tensor_copy(
                        y_sb[:, offs[t]:offs[t] + tiles[t]],
                        psy[t][:, :tiles[t]],
                    )
                    # per-tile store (act ring: free in stage 2, and the
                    # final DMA after the last matmul stays small)
                    nc.scalar.dma_start(
                        yT[md, :, offs[t]:offs[t] + tiles[t]],
                        y_sb[:, offs[t]:offs[t] + tiles[t]],
                    )

    nc.compile()
    return nc


def _pack_w1(wg: np.ndarray, wu: np.ndarray) -> np.ndarray:
    """[D, F] x2 -> [P, MF, KD, 2, P] bf16, matching the kernel's layout."""
    # w1[p, mf, kd, gu, c] = w_gu[kd*128 + p, mf*128 + c]
    stack = np.stack([wg, wu], axis=0)            # [2, D, F]
    r = stack.reshape(2, KD, P, MF, P)            # [gu, kd, p, mf, c]
    return np.ascontiguousarray(r.transpose(2, 3, 1, 0, 4)).astype(BF16)


def _pack_wd(wd: np.ndarray) -> np.ndarray:
    """[F, D] -> [P, KD, MF, P] bf16. wdp[p, md, kf, c] = wd[kf*128+p, md*128+c]"""
    r = wd.reshape(MF, P, KD, P)                  # [kf, p, md, c]
    return np.ascontiguousarray(r.transpose(1, 2, 0, 3)).astype(BF16)


def _run_device(in_maps, tiles):
    from concourse.bass_utils import run_bass_kernel_spmd

    if tiles not in _program_cache:
        _program_cache[tiles] = _build_program(tiles)
    nc = _program_cache[tiles]
    res = run_bass_kernel_spmd(nc, in_maps, core_ids=list(range(N_CORES)))
    return [r["yT"] for r in res.results]


def kernel(hidden_states, routing_mask, wg0, wu0, wd0, wg1, wu1, wd1,
           _run=None):
    hidden_states = np.asarray(hidden_states, dtype=np.float32)
    routing_mask = np.asarray(routing_mask)
    B, S, D = hidden_states.shape
    NTOK = B * S
    x = hidden_states.reshape(NTOK, D)
    mask = routing_mask.reshape(NTOK)

    idx = [np.nonzero(mask == e)[0] for e in (0, 1)]
    maxpc = max(
        (len(idx[0]) + CPE - 1) // CPE,
        (len(idx[1]) + CPE - 1) // CPE,
        1,
    )
    tiles = _token_tiles(maxpc)
    C = sum(tiles)

    w1_packed = [_pack_w1(np.asarray(wg0), np.asarray(wu0)),
                 _pack_w1(np.asarray(wg1), np.asarray(wu1))]
    wd_packed = [_pack_wd(np.asarray(wd0)), _pack_wd(np.asarray(wd1))]

    in_maps = []
    chunks = []  # (expert, token_indices) per core
    for core in range(N_CORES):
        e = core // CPE
        slot = core % CPE
        ids = idx[e]
        # split ids into CPE nearly-equal chunks
        bounds = [(len(ids) * i) // CPE for i in range(CPE + 1)]
        ids_c = ids[bounds[slot]:bounds[slot + 1]]
        chunks.append((e, ids_c))

        xc = np.zeros((C, D), dtype=np.float32)
        xc[: len(ids_c)] = x[ids_c]
        # xT[p, kd, c] = xc[c, kd*128 + p]
        xT = np.ascontiguousarray(
            xc.reshape(C, KD, P).transpose(2, 1, 0)
        ).astype(BF16)
        in_maps.append({
            "xT": xT,
            "w1": w1_packed[e],
            "wd": wd_packed[e],
        })

    run = _run if _run is not None else _run_device
    outs = run(in_maps, tiles)

    y_full = np.zeros((NTOK, D), dtype=np.float32)
    for core in range(N_CORES):
        _, ids_c = chunks[core]
        if len(ids_c) == 0:
            continue
        yT = np.asarray(outs[core]).astype(np.float32).reshape(D, C)
        y_full[ids_c] = yT[:, : len(ids_c)].T
    return y_full.reshape(B, S, D)


# revision 28
# speedup vs baseline: 1.0080x; 1.0064x over previous
"""MoE (2-expert SwiGLU) Trainium2 kernel, 8-core SPMD.

Strategy: since the MLPs have no biases and silu(0) = 0, MLP(0) = 0, so each
token only needs the expert it is routed to.  The host gathers tokens by
expert (MoE dispatch), cores 0-3 process expert-0 tokens and cores 4-7
expert-1 tokens (~1/8 of total tokens per core), each core running a dense
SwiGLU MLP with its expert's weights.  The host scatters per-core outputs
back into the full [B, S, D] output.  This halves FLOPs vs. the reference's
dense-masked formulation and needs no collectives.

Device dataflow (per core, transposed so no on-chip transposes are needed):
  yT = Wd^T @ (silu(Wg^T @ xT) * (Wu^T @ xT))
Weights are the stationary matmul operand, token-columns the moving operand.
All matmuls are bf16 with fp32 PSUM accumulation.  The FF intermediate `h`
for all of a core's tokens stays resident in SBUF, so each weight byte is
DMA'd exactly once per core.
"""

import sys

for _p in ("/opt/trn_rl_repo", "/root/.axon_site/_ro/trn_rl_repo"):
    if _p not in sys.path:
        sys.path.append(_p)

import numpy as np
import ml_dtypes

BF16 = ml_dtypes.bfloat16

D_MODEL = 1024
D_FF = 4096
P = 128
KD = D_MODEL // P  # 8   k-tiles over d_model
MF = D_FF // P     # 32  tiles over d_ff
N_CORES = 8
CPE = 4            # cores per expert
NT = 3             # token tiles per core

_program_cache: dict[tuple, object] = {}


def _token_tiles(maxpc: int) -> tuple:
    """Split the per-core token count into NT near-equal tiles.  Tile
    offsets stay multiples of 8 (16B-aligned bf16 slices); the last tile
    absorbs the remainder.  Each tile <= 512 (PSUM bank limit)."""
    C = max(maxpc, 24)
    t = 8 * ((C + NT * 8 - 1) // (NT * 8))    # per-tile, rounded up to 8
    tiles = [t] * (NT - 1) + [C - t * (NT - 1)]
    assert all(0 < s <= 512 for s in tiles), (maxpc, tiles)
    return tuple(tiles)


def _build_program(tiles: tuple):
    """Bass program for one core: x [D,C] -> y [D,C], C = sum(tiles) tokens."""
    import concourse.tile as tile
    from concourse import mybir, bacc

    C = sum(tiles)
    offs = [sum(tiles[:i]) for i in range(len(tiles))]
    TSMAX = max(tiles)
    f32 = mybir.dt.float32
    b16 = mybir.dt.bfloat16

    nc = bacc.Bacc()
    xT = nc.declare_dram_parameter("xT", [P, KD, C], b16, isOutput=False)
    # w1[mf, p, gu, kd, c] = (wg if gu==0 else wu)[kd*128 + p, mf*128 + c]
    w1 = nc.declare_dram_parameter("w1", [MF, P, 2, KD, P], b16, isOutput=False)
    # wdp[md, p, kf, c] = wd[kf*128 + p, md*128 + c]
    wdp = nc.declare_dram_parameter("wd", [KD, P, MF, P], b16, isOutput=False)
    yT = nc.declare_dram_parameter("yT", [KD, P, C], b16, isOutput=True)

    with tile.TileContext(nc) as tc:
        with (
            tc.tile_pool(name="xp", bufs=1) as xp,
            tc.tile_pool(name="hp", bufs=1) as hp,
            tc.tile_pool(name="w1p", bufs=3) as w1p,
            tc.tile_pool(name="wdpool", bufs=2) as wdpool,
            tc.tile_pool(name="silp", bufs=4) as silp,
            tc.tile_pool(name="yp", bufs=2) as yp,
            # one PSUM pool for BOTH stages: stage 2 allocates from the
            # same tags stage 1 used, so its tiles only wait on their own
            # slot's last reader instead of a pool-close barrier.
            tc.tile_pool(name="ps1", bufs=NT, space="PSUM") as ps1,
        ):
            x_sb = xp.tile([P, KD, C], b16)
            h_sb = hp.tile([P, MF, C], b16)
            # The sync HWDGE ring drains in issue order, so stage the startup
            # transfers in the order the PE consumes them: first weight tile,
            # then x k-slices in growing chunks.
            wt0 = w1p.tile([P, 2, KD, P], b16, tag="wt", name="wt_0")
            nc.sync.dma_start(wt0[:], w1[0])
            nc.sync.dma_start(x_sb[:, 0], xT[:, 0])
            nc.sync.dma_start(x_sb[:, 1], xT[:, 1])
            nc.sync.dma_start(x_sb[:, 2:4], xT[:, 2:4])
            nc.sync.dma_start(x_sb[:, 4:], xT[:, 4:])

            # Stage 1: h = silu(Wg^T x) * (Wu^T x), laid out [ff-part, C]
            for mf in range(MF):
                if mf == 0:
                    wt = wt0
                else:
                    wt = w1p.tile([P, 2, KD, P], b16, tag="wt",
                                  name=f"wt_{mf}")
                    nc.sync.dma_start(wt[:], w1[mf])
                psg = [ps1.tile([P, 512], f32, tag="psg", name=f"psg_{mf}_{t}")
                       for t in range(NT)]
                psu = [ps1.tile([P, 512], f32, tag="psu", name=f"psu_{mf}_{t}")
                       for t in range(NT)]
                for kd in range(KD):
                    for gu in range(2):
                        ps = psg if gu == 0 else psu
                        for t in range(NT):
                            nc.tensor.matmul(
                                ps[t][:, :tiles[t]],
                                wt[:, gu, kd],
                                x_sb[:, kd, offs[t]:offs[t] + tiles[t]],
                                start=(kd == 0),
                                stop=(kd == KD - 1),
                            )
                for t in range(NT):
                    sil = silp.tile([P, TSMAX], f32, tag="sil",
                                    name=f"sil_{mf}_{t}")
                    nc.scalar.activation(
                        sil[:, :tiles[t]], psg[t][:, :tiles[t]],
                        mybir.ActivationFunctionType.Silu,
                    )
                    nc.vector.tensor_mul(
                        h_sb[:, mf, offs[t]:offs[t] + tiles[t]],
                        sil[:, :tiles[t]], psu[t][:, :tiles[t]],
                    )

            # Stage 2: y = Wd^T h, laid out [d-part, C].  psy reuses the
            # stage-1 PSUM tags (alternating per md -> 2-md slot-reuse
            # distance, and the first md only waits on stage-1's last
            # ACT/TT instead of a barrier).
            for md in range(KD):
                wdt = wdpool.tile([P, MF, P], b16)
                nc.sync.dma_start(wdt[:], wdp[md])
                y_sb = yp.tile([P, C], b16)
                ptag = "psg" if md % 2 == 0 else "psu"
                psy = [ps1.tile([P, 512], f32, tag=ptag, name=f"psy_{md}_{t}")
                       for t in range(NT)]
                for kf in range(MF):
                    for t in range(NT):
                        nc.tensor.matmul(
                            psy[t][:, :tiles[t]],
                            wdt[:, kf],
                            h_sb[:, kf, offs[t]:offs[t] + tiles[t]],
                            start=(kf == 0),
                            stop=(kf == MF - 1),
                        )
                for t in range(NT):
                    nc.vector.tensor_copy(
                        y_sb[:, offs[t]:offs[t] + tiles[t]],
                        psy[t][:, :tiles[t]],
                    )
                    # per-tile store on the act ring (idle in stage 2;
                    # keeps the final DMA after the last matmul small and
                    # off the wd-load ring)
                    nc.scalar.dma_start(
                        yT[md, :, offs[t]:offs[t] + tiles[t]],
                        y_sb[:, offs[t]:offs[t] + tiles[t]],
                    )

    nc.compile()
    return nc


def _pack_w1(wg: np.ndarray, wu: np.ndarray) -> np.ndarray:
    """[D, F] x2 -> [MF, P, 2, KD, P] bf16, matching the kernel's layout."""
    # w1[mf, p, gu, kd, c] = w_gu[kd*128 + p, mf*128 + c]
    stack = np.stack([wg, wu], axis=0)            # [2, D, F]
    r = stack.reshape(2, KD, P, MF, P)            # [gu, kd, p, mf, c]
    return np.ascontiguousarray(r.transpose(3, 2, 0, 1, 4)).astype(BF16)


def _pack_wd(wd: np.ndarray) -> np.ndarray:
    """[F, D] -> [KD, P, MF, P] bf16. wdp[md, p, kf, c] = wd[kf*128+p, md*128+c]"""
    r = wd.reshape(MF, P, KD, P)                  # [kf, p, md, c]
    return np.ascontiguousarray(r.transpose(2, 1, 0, 3)).astype(BF16)


def _run_device(in_maps, tiles):
    from concourse.bass_utils import run_bass_kernel_spmd

    if tiles not in _program_cache:
        _program_cache[tiles] = _build_program(tiles)
    nc = _program_cache[tiles]
    res = run_bass_kernel_spmd(nc, in_maps, core_ids=list(range(N_CORES)))
    return [r["yT"] for r in res.results]


def kernel(hidden_states, routing_mask, wg0, wu0, wd0, wg1, wu1, wd1,
           _run=None):
    hidden_states = np.asarray(hidden_states, dtype=np.float32)
    routing_mask = np.asarray(routing_mask)
    B, S, D = hidden_states.shape
    NTOK = B * S
    x = hidden_states.reshape(NTOK, D)
    mask = routing_mask.reshape(NTOK)

    idx = [np.nonzero(mask == e)[0] for e in (0, 1)]
    maxpc = max(
        (len(idx[0]) + CPE - 1) // CPE,
        (len(idx[1]) + CPE - 1) // CPE,
        1,
    )
    tiles = _token_tiles(maxpc)
    C = sum(tiles)

    w1_packed = [_pack_w1(np.asarray(wg0), np.asarray(wu0)),
                 _pack_w1(np.asarray(wg1), np.asarray(wu1))]
    wd_packed = [_pack_wd(np.asarray(wd0)), _pack_wd(np.asarray(wd1))]

    in_maps = []
    chunks = []  # (expert, token_indices) per core
    for core in range(N_CORES):
        e = core // CPE
        slot = core % CPE
        ids = idx[e]
        # split ids into CPE nearly-equal chunks
        bounds = [(len(ids) * i) // CPE for i in range(CPE + 1)]
        ids_c = ids[bounds[slot]:bounds[slot + 1]]
        chunks.append((e, ids_c))

        xc = np.zeros((C, D), dtype=np.float32)
        xc[: len(ids_c)] = x[ids_c]
        # xT[p, kd, c] = xc[c, kd*128 + p]
        xT = np.ascontiguousarray(
            xc.reshape(C, KD, P).transpose(2, 1, 0)
        ).astype(BF16)
        in_maps.append({
            "xT": xT,
            "w1": w1_packed[e],
            "wd": wd_packed[e],
        })

    run = _run if _run is not None else _run_device
    outs = run(in_maps, tiles)

    y_full = np.zeros((NTOK, D), dtype=np.float32)
    for core in range(N_CORES):
        _, ids_c = chunks[core]
        if len(ids_c) == 0:
            continue
        yT = np.asarray(outs[core]).astype(np.float32).reshape(D, C)
        y_full[ids_c] = yT[:, : len(ids_c)].T
    return y_full.reshape(B, S, D)


# revision 29
# speedup vs baseline: 1.0149x; 1.0068x over previous
"""MoE (2-expert SwiGLU) Trainium2 kernel, 8-core SPMD.

Strategy: since the MLPs have no biases and silu(0) = 0, MLP(0) = 0, so each
token only needs the expert it is routed to.  The host gathers tokens by
expert (MoE dispatch), cores 0-3 process expert-0 tokens and cores 4-7
expert-1 tokens (~1/8 of total tokens per core), each core running a dense
SwiGLU MLP with its expert's weights.  The host scatters per-core outputs
back into the full [B, S, D] output.  This halves FLOPs vs. the reference's
dense-masked formulation and needs no collectives.

Device dataflow (per core, transposed so no on-chip transposes are needed):
  yT = Wd^T @ (silu(Wg^T @ xT) * (Wu^T @ xT))
Weights are the stationary matmul operand, token-columns the moving operand.
All matmuls are bf16 with fp32 PSUM accumulation.  The FF intermediate `h`
for all of a core's tokens stays resident in SBUF, so each weight byte is
DMA'd exactly once per core.
"""

import sys

for _p in ("/opt/trn_rl_repo", "/root/.axon_site/_ro/trn_rl_repo"):
    if _p not in sys.path:
        sys.path.append(_p)

import numpy as np
import ml_dtypes

BF16 = ml_dtypes.bfloat16

D_MODEL = 1024
D_FF = 4096
P = 128
KD = D_MODEL // P  # 8   k-tiles over d_model
MF = D_FF // P     # 32  tiles over d_ff
N_CORES = 8
CPE = 4            # cores per expert
NT = 3             # token tiles per core

_program_cache: dict[tuple, object] = {}


def _token_tiles(maxpc: int) -> tuple:
    """Split the per-core token count into NT near-equal tiles.  Tile
    offsets stay multiples of 8 (16B-aligned bf16 slices); the last tile
    absorbs the remainder.  Each tile <= 512 (PSUM bank limit)."""
    C = max(maxpc, 24)
    t = 8 * ((C + NT * 8 - 1) // (NT * 8))    # per-tile, rounded up to 8
    tiles = [t] * (NT - 1) + [C - t * (NT - 1)]
    assert all(0 < s <= 512 for s in tiles), (maxpc, tiles)
    return tuple(tiles)


def _build_program(tiles: tuple):
    """Bass program for one core: x [D,C] -> y [D,C], C = sum(tiles) tokens."""
    import concourse.tile as tile
    from concourse import mybir, bacc

    C = sum(tiles)
    offs = [sum(tiles[:i]) for i in range(len(tiles))]
    TSMAX = max(tiles)
    f32 = mybir.dt.float32
    b16 = mybir.dt.bfloat16

    nc = bacc.Bacc()
    xT = nc.declare_dram_parameter("xT", [P, KD, C], b16, isOutput=False)
    # w1[mf, p, gu, kd, c] = (wg if gu==0 else wu)[kd*128 + p, mf*128 + c]
    w1 = nc.declare_dram_parameter("w1", [MF, P, 2, KD, P], b16, isOutput=False)
    # wdp[md, p, kf, c] = wd[kf*128 + p, md*128 + c]
    wdp = nc.declare_dram_parameter("wd", [KD, P, MF, P], b16, isOutput=False)
    yT = nc.declare_dram_parameter("yT", [KD, P, C], b16, isOutput=True)

    with tile.TileContext(nc) as tc:
        with (
            tc.tile_pool(name="xp", bufs=1) as xp,
            tc.tile_pool(name="hp", bufs=1) as hp,
            tc.tile_pool(name="w1p", bufs=3) as w1p,
            tc.tile_pool(name="wdpool", bufs=2) as wdpool,
            tc.tile_pool(name="silp", bufs=4) as silp,
            tc.tile_pool(name="yp", bufs=2) as yp,
            # one PSUM pool for BOTH stages: stage 2 allocates from the
            # same tags stage 1 used, so its tiles only wait on their own
            # slot's last reader instead of a pool-close barrier.
            tc.tile_pool(name="ps1", bufs=NT, space="PSUM") as ps1,
        ):
            x_sb = xp.tile([P, KD, C], b16)
            h_sb = hp.tile([P, MF, C], b16)
            # The sync HWDGE ring drains in issue order, so stage the startup
            # transfers in the order the PE consumes them: first weight tile,
            # then x k-slices in growing chunks.
            wt0 = w1p.tile([P, 2, KD, P], b16, tag="wt", name="wt_0")
            nc.sync.dma_start(wt0[:], w1[0])
            nc.sync.dma_start(x_sb[:, 0], xT[:, 0])
            nc.sync.dma_start(x_sb[:, 1], xT[:, 1])
            nc.sync.dma_start(x_sb[:, 2:4], xT[:, 2:4])
            nc.sync.dma_start(x_sb[:, 4:], xT[:, 4:])

            # Stage 1: h = silu(Wg^T x) * (Wu^T x), laid out [ff-part, C]
            for mf in range(MF):
                if mf == 0:
                    wt = wt0
                else:
                    wt = w1p.tile([P, 2, KD, P], b16, tag="wt",
                                  name=f"wt_{mf}")
                    nc.sync.dma_start(wt[:], w1[mf])
                psg = [ps1.tile([P, 512], f32, tag="psg", name=f"psg_{mf}_{t}")
                       for t in range(NT)]
                psu = [ps1.tile([P, 512], f32, tag="psu", name=f"psu_{mf}_{t}")
                       for t in range(NT)]
                if mf == 0:
                    # kd-major: consume x k-slices in DMA-arrival order
                    order = [(kd, gu, t) for kd in range(KD)
                             for gu in range(2) for t in range(NT)]
                else:
                    # t-major: each PSUM tile's 8-kd chain runs
                    # consecutively, so the six per-mf slot-wait matmuls
                    # spread out (the embedded semaphore waits hide in the
                    # stream) and tile 0's silu/mul starts earlier.
                    order = [(kd, gu, t) for t in range(NT)
                             for gu in range(2) for kd in range(KD)]
                for kd, gu, t in order:
                    ps = psg if gu == 0 else psu
                    nc.tensor.matmul(
                        ps[t][:, :tiles[t]],
                        wt[:, gu, kd],
                        x_sb[:, kd, offs[t]:offs[t] + tiles[t]],
                        start=(kd == 0),
                        stop=(kd == KD - 1),
                    )
                for t in range(NT):
                    sil = silp.tile([P, TSMAX], f32, tag="sil",
                                    name=f"sil_{mf}_{t}")
                    nc.scalar.activation(
                        sil[:, :tiles[t]], psg[t][:, :tiles[t]],
                        mybir.ActivationFunctionType.Silu,
                    )
                    nc.vector.tensor_mul(
                        h_sb[:, mf, offs[t]:offs[t] + tiles[t]],
                        sil[:, :tiles[t]], psu[t][:, :tiles[t]],
                    )

            # Stage 2: y = Wd^T h, laid out [d-part, C].  psy reuses the
            # stage-1 PSUM tags (alternating per md -> 2-md slot-reuse
            # distance, and the first md only waits on stage-1's last
            # ACT/TT instead of a barrier).
            for md in range(KD):
                wdt = wdpool.tile([P, MF, P], b16)
                nc.sync.dma_start(wdt[:], wdp[md])
                y_sb = yp.tile([P, C], b16)
                ptag = "psg" if md % 2 == 0 else "psu"
                psy = [ps1.tile([P, 512], f32, tag=ptag, name=f"psy_{md}_{t}")
                       for t in range(NT)]
                for kf in range(MF):
                    for t in range(NT):
                        nc.tensor.matmul(
                            psy[t][:, :tiles[t]],
                            wdt[:, kf],
                            h_sb[:, kf, offs[t]:offs[t] + tiles[t]],
                            start=(kf == 0),
                            stop=(kf == MF - 1),
                        )
                for t in range(NT):
                    nc.vector.tensor_copy(
                        y_sb[:, offs[t]:offs[t] + tiles[t]],
                        psy[t][:, :tiles[t]],
                    )
                    # per-tile store on the act ring (idle in stage 2;
                    # keeps the final DMA after the last matmul small and
                    # off the wd-load ring)
                    nc.scalar.dma_start(
                        yT[md, :, offs[t]:offs[t] + tiles[t]],
                        y_sb[:, offs[t]:offs[t] + tiles[t]],
                    )

    nc.compile()
    return nc


def _pack_w1(wg: np.ndarray, wu: np.ndarray) -> np.ndarray:
    """[D, F] x2 -> [MF, P, 2, KD, P] bf16, matching the kernel's layout."""
    # w1[mf, p, gu, kd, c] = w_gu[kd*128 + p, mf*128 + c]
    stack = np.stack([wg, wu], axis=0)            # [2, D, F]
    r = stack.reshape(2, KD, P, MF, P)            # [gu, kd, p, mf, c]
    return np.ascontiguousarray(r.transpose(3, 2, 0, 1, 4)).astype(BF16)


def _pack_wd(wd: np.ndarray) -> np.ndarray:
    """[F, D] -> [KD, P, MF, P] bf16. wdp[md, p, kf, c] = wd[kf*128+p, md*128+c]"""
    r = wd.reshape(MF, P, KD, P)                  # [kf, p, md, c]
    return np.ascontiguousarray(r.transpose(2, 1, 0, 3)).astype(BF16)


def _run_device(in_maps, tiles):
    from concourse.bass_utils import run_bass_kernel_spmd

    if tiles not in _program_cache:
        _program_cache[tiles] = _build_program(tiles)
    nc = _program_cache[tiles]
    res = run_bass_kernel_spmd(nc, in_maps, core_ids=list(range(N_CORES)))
    return [r["yT"] for r in res.results]


def kernel(hidden_states, routing_mask, wg0, wu0, wd0, wg1, wu1, wd1,
           _run=None):
    hidden_states = np.asarray(hidden_states, dtype=np.float32)
    routing_mask = np.asarray(routing_mask)
    B, S, D = hidden_states.shape
    NTOK = B * S
    x = hidden_states.reshape(NTOK, D)
    mask = routing_mask.reshape(NTOK)

    idx = [np.nonzero(mask == e)[0] for e in (0, 1)]
    maxpc = max(
        (len(idx[0]) + CPE - 1) // CPE,
        (len(idx[1]) + CPE - 1) // CPE,
        1,
    )
    tiles = _token_tiles(maxpc)
    C = sum(tiles)

    w1_packed = [_pack_w1(np.asarray(wg0), np.asarray(wu0)),
                 _pack_w1(np.asarray(wg1), np.asarray(wu1))]
    wd_packed = [_pack_wd(np.asarray(wd0)), _pack_wd(np.asarray(wd1))]

    in_maps = []
    chunks = []  # (expert, token_indices) per core
    for core in range(N_CORES):
        e = core // CPE
        slot = core % CPE
        ids = idx[e]
        # split ids into CPE nearly-equal chunks
        bounds = [(len(ids) * i) // CPE for i in range(CPE + 1)]
        ids_c = ids[bounds[slot]:bounds[slot + 1]]
        chunks.append((e, ids_c))

        xc = np.zeros((C, D), dtype=np.float32)
        xc[: len(ids_c)] = x[ids_c]
        # xT[p, kd, c] = xc[c, kd*128 + p]
        xT = np.ascontiguousarray(
            xc.reshape(C, KD, P).transpose(2, 1, 0)
        ).astype(BF16)
        in_maps.append({
            "xT": xT,
            "w1": w1_packed[e],
            "wd": wd_packed[e],
        })

    run = _run if _run is not None else _run_device
    outs = run(in_maps, tiles)

    y_full = np.zeros((NTOK, D), dtype=np.float32)
    for core in range(N_CORES):
        _, ids_c = chunks[core]
        if len(ids_c) == 0:
            continue
        yT = np.asarray(outs[core]).astype(np.float32).reshape(D, C)
        y_full[ids_c] = yT[:, : len(ids_c)].T
    return y_full.reshape(B, S, D)
